# revision 2
# baseline (speedup 1.0000x reference)
"""Trainium2 Bass kernel for nn_Attention_26336739459136 — v2.

Changes vs v1 baseline (625us):
  - all streamed inputs bf16 (host-converted): input DMA 22MB -> 11MB/core;
    projection matmuls bf16 (237ns/MM vs fp32r 249).
  - phase 0 reordered k-proj -> AG#1a/b -> value' -> AG#2 -> q-proj: the
    collectives (3 serialized ops, ~180us wire total) start ~70us earlier,
    so phase 1 never stalls on AG#1b and phase 2 never waits on AG#2.
  - wps/wpq prefetch moved to the vector DMA queue so the kout stores
    (which gate AG#1a) aren't queued behind 4MB of weight prefetch.
  - q-row denominator blocks (scores of own q-rows vs all q-keys, exp'd
    and row-summed only) use fp8e4 DoubleRow matmuls: 2 k-subtiles per MM,
    halving those chains from 8 MMs to 4.  Numerics sim: x_q rel err
    7.1e-3 (vs 2e-2 gate); the t-row blocks stay bf16 (fp8 there measured
    1.9e-2 -- too close).  q8/k8 are fp8 copies of the UNSCALED q/k (the
    1/32 score scale is applied at the exp activation input instead;
    scaled-q values ~0.04 would land in fp8 subnormals).

Sharding: unchanged -- every core owns an interleaved slice of 512 s-rows
+ 512 q-rows; SPMD identical on all 8 cores.
"""

import numpy as np

import concourse.bass as bass
import concourse.bacc as bacc
import concourse.mybir as mybir
import concourse.tile as tile
from concourse.bass_utils import run_bass_kernel_spmd

P = 128
C = 1024
D = 1024
NCORES = 8
HALF = 512
ROWS = 1024
SCALE2 = float(C) ** -0.5

F32 = mybir.dt.float32
BF16 = mybir.dt.bfloat16
E4 = mybir.dt.float8e4
AX = mybir.AxisListType.X
EXP = mybir.ActivationFunctionType.Exp
COPY = mybir.ActivationFunctionType.Copy
DR = mybir.MatmulPerfMode.DoubleRow

SCORE_DT = BF16


def _r(ap):
    """[N*128, F] dram view -> [128, N, F] partition-major tiles."""
    return ap.rearrange("(o p) f -> p o f", p=P)


def build_program():
    nc = bacc.Bacc(
        "TRN2", target_bir_lowering=False, debug=False, num_devices=NCORES
    )

    # ---- I/O (all streamed inputs bf16) ----
    # aT columns 0:512 = (s+TG_prompt).T slice, 512:1024 = query.T slice
    aT = nc.dram_tensor("aT", [C, ROWS], BF16, kind="ExternalInput")
    # raw s.T slice -- the value' s-branch needs s, not t
    sT = nc.dram_tensor("sT", [C, HALF], BF16, kind="ExternalInput")
    # host-packed: wqkR[p, dt*1024 + ct*128 + f] = W_qkv.T[ct*128+p, dt*128+f]
    wqkR = nc.dram_tensor("wqkR", [P, 16 * C], BF16, kind="ExternalInput")
    wpsT = nc.dram_tensor("wpsT", [D, D], BF16, kind="ExternalInput")
    wpqT = nc.dram_tensor("wpqT", [D, D], BF16, kind="ExternalInput")
    out_s = nc.dram_tensor("out_s", [HALF, D], F32, kind="ExternalOutput")
    out_q = nc.dram_tensor("out_q", [HALF, D], F32, kind="ExternalOutput")

    # ---- collective buffers ----
    # A/B d-halves in separate tensors (per-tensor dependency tracking)
    kt_inA = nc.dram_tensor("kt_inA", [D // 2, ROWS], SCORE_DT, kind="Internal")
    kt_inB = nc.dram_tensor("kt_inB", [D // 2, ROWS], SCORE_DT, kind="Internal")
    kt_allA = nc.dram_tensor(
        "kt_allA", [NCORES * (D // 2), ROWS], SCORE_DT, kind="Internal",
        addr_space="Shared"
    )
    kt_allB = nc.dram_tensor(
        "kt_allB", [NCORES * (D // 2), ROWS], SCORE_DT, kind="Internal",
        addr_space="Shared"
    )
    v_in = nc.dram_tensor("v_in", [ROWS, D], BF16, kind="Internal")
    v_all = nc.dram_tensor(
        "v_all", [NCORES * ROWS, D], BF16, kind="Internal", addr_space="Shared"
    )

    with tile.TileContext(nc) as tc:
        with tc.tile_pool(name="persist", bufs=1) as persist:
            # qT [d, i] own 1024 rows, bf16, SCALE2 folded (numerator use)
            qT = persist.tile([P, D // P, ROWS], SCORE_DT)
            # q8: fp8 of UNSCALED q for own q-rows (i in [512,1024))
            q8 = persist.tile([P, D // P, HALF], E4)
            dparts = persist.tile([P, 8 * NCORES], F32)
            ones_sb = persist.tile([P, 1], BF16)
            nc.vector.memset(ones_sb[:], 1.0)

            # ================= phase 0: projections =================
            with (
                tc.tile_pool(name="ph0", bufs=1) as ph0,
                tc.tile_pool(name="ph0w", bufs=3) as ph0w,
                tc.tile_pool(name="ph0s", bufs=16) as ph0s,
                tc.tile_pool(name="psum0", bufs=3, space="PSUM") as psum0,
            ):
                # aT already holds [t | query] (t added on host): the
                # sync queue carries just 4MB before wps/wpq
                aT_sb = ph0.tile([P, C // P, ROWS], BF16)
                for ct in range(C // P):
                    nc.sync.dma_start(
                        aT_sb[:, ct], aT[ct * P : (ct + 1) * P, :]
                    )
                wps_sb = ph0.tile([P, C // P, D], BF16)
                wpq_sb = ph0.tile([P, C // P, D], BF16)
                sT_sb = ph0.tile([P, C // P, HALF], BF16)

                def rhs_c(ct, ic):
                    # ic=0: t-rows (cols 0:HALF), ic=1: query rows
                    return aT_sb[:, ct, ic * HALF : ic * HALF + HALF]

                def fetch_w(dt_i):
                    wt = ph0w.tile([P, C], BF16, tag="wqk", bufs=17,
                                   name=f"wt{dt_i}")
                    nc.scalar.dma_start(
                        wt[:], wqkR[:, dt_i * C : (dt_i + 1) * C]
                    )
                    return wt

                # issue every weight fetch NOW: dma_start instructions sit
                # in the scalar FIFO ahead of any compute-dependent copies,
                # so the DMA engine streams all 4MB immediately
                all_wts = {}
                for dt_i in list(range(D // P, 2 * D // P)) + list(range(D // P)):
                    all_wts[dt_i] = fetch_w(dt_i)
                nc.scalar.dma_start(wps_sb[:], _r(wpsT[:]))
                nc.scalar.dma_start(wpq_sb[:], _r(wpqT[:]))

                def run_chain(dt_i, ic, wt):
                    ps = psum0.tile([P, HALF], F32, tag="ps0")
                    for ct in range(C // P):
                        nc.tensor.matmul(
                            ps[:],
                            (wt[:, ct * P : (ct + 1) * P]),
                            (rhs_c(ct, ic)),
                            start=(ct == 0),
                            stop=(ct == C // P - 1),
                        )
                    if dt_i < D // P:  # q: bf16 scaled + fp8 raw (q-rows)
                        nc.scalar.activation(
                            qT[:, dt_i, ic * HALF : (ic + 1) * HALF],
                            ps[:],
                            COPY,
                            scale=SCALE2,
                        )
                        if ic == 1:
                            nc.scalar.copy(q8[:, dt_i], ps[:])
                    else:  # k: bf16 out, stream to kt_inA/B d-halves
                        kout = ph0s.tile([P, HALF], SCORE_DT, tag="kout")
                        nc.scalar.copy(kout[:], ps[:])
                        kd = dt_i - D // P
                        kt_dst = kt_inA if kd < D // P // 2 else kt_inB
                        kdh = kd % (D // P // 2)
                        nc.sync.dma_start(
                            kt_dst[
                                kdh * P : (kdh + 1) * P,
                                ic * HALF : (ic + 1) * HALF,
                            ],
                            kout[:],
                        )

                def proj_chain(dt_i):
                    wt = all_wts[dt_i]
                    for ic in (1, 0):
                        run_chain(dt_i, ic, wt)

                # ---- k projection first: A d-half, then B d-half ----
                KD0 = D // P
                for dt_i in range(KD0, KD0 + 4):
                    proj_chain(dt_i)
                nc.gpsimd.collective_compute(
                    "AllGather",
                    mybir.AluOpType.bypass,
                    replica_groups=[list(range(NCORES))],
                    ins=[kt_inA[:].opt()],
                    outs=[kt_allA[:].opt()],
                )
                for dt_i in range(KD0 + 4, KD0 + 8):
                    proj_chain(dt_i)
                nc.gpsimd.collective_compute(
                    "AllGather",
                    mybir.AluOpType.bypass,
                    replica_groups=[list(range(NCORES))],
                    ins=[kt_inB[:].opt()],
                    outs=[kt_allB[:].opt()],
                )


                # raw sT streams after the kouts on the sync queue
                for ct in range(C // P):
                    nc.sync.dma_start(
                        sT_sb[:, ct], sT[ct * P : (ct + 1) * P, :]
                    )
                # ---- q projection (phase 1 gate) ----
                for dt_i in range(D // P):
                    proj_chain(dt_i)

                # ---- value' shards (RAW s/query slices), then AG#2 ----
                vall_sb = ph0.tile([P, 8, D], BF16)
                for half_i, w_sb in enumerate([wps_sb, wpq_sb]):
                    lo = HALF if half_i == 0 else 0
                    for ec in range(2):
                        for jt in range(HALF // P):
                            ps = psum0.tile([P, HALF], F32, tag="ps0")
                            src_sb = aT_sb if half_i == 0 else sT_sb
                            for ct in range(C // P):
                                nc.tensor.matmul(
                                    ps[:],
                                    (src_sb[:, ct, lo + jt * P : lo + (jt + 1) * P]),
                                    (w_sb[:, ct, ec * HALF : (ec + 1) * HALF]),
                                    start=(ct == 0),
                                    stop=(ct == C // P - 1),
                                )
                            nc.scalar.copy(
                                vall_sb[
                                    :,
                                    half_i * 4 + jt,
                                    ec * HALF : (ec + 1) * HALF,
                                ],
                                ps[:],
                            )
                nc.sync.dma_start(_r(v_in[:]), vall_sb[:])
                nc.gpsimd.collective_compute(
                    "AllGather",
                    mybir.AluOpType.bypass,
                    replica_groups=[list(range(NCORES))],
                    ins=[v_in[:].opt()],
                    outs=[v_all[:].opt()],
                )

            # ================= phases 1+2 =================
            with tc.tile_pool(name="epool", bufs=1) as epool:
                eC = epool.tile([P, 32, HALF], BF16)
                eD = epool.tile([P, 32, HALF], BF16)
                phase_12(nc, tc, qT, q8, eC, eD, dparts, ones_sb, kt_allA,
                         kt_allB, v_all, out_s, out_q)
    nc.compile()
    return nc


def phase_12(nc, tc, qT, q8, eC, eD, dparts, ones_sb, kt_allA, kt_allB,
             v_all, out_s, out_q):
            # ================= phase 1: scores + exp =================
            with (
                tc.tile_pool(name="kt", bufs=2) as ktp,
                tc.tile_pool(name="sc", bufs=4) as scp,
                tc.tile_pool(name="psum1", bufs=8, space="PSUM") as psum1,
            ):
                HD = D // P // 2
                for r in range(NCORES):
                    ktA = ktp.tile([P, HD, ROWS], SCORE_DT, tag="ktA")
                    ktB = ktp.tile([P, HD, ROWS], SCORE_DT, tag="ktB")
                    nc.sync.dma_start(
                        ktA[:], _r(kt_allA[r * (D // 2) : (r + 1) * (D // 2), :])
                    )
                    nc.sync.dma_start(
                        ktB[:], _r(kt_allB[r * (D // 2) : (r + 1) * (D // 2), :])
                    )

                    def kt(dd):
                        return ktA[:, dd] if dd < HD else ktB[:, dd - HD]

                    k8A = ktp.tile([P, HD, HALF], E4, tag="k8A")
                    k8B = ktp.tile([P, HD, HALF], E4, tag="k8B")
                    nc.vector.tensor_copy(k8A[:], ktA[:, :, HALF:ROWS])
                    nc.vector.tensor_copy(k8B[:], ktB[:, :, HALF:ROWS])

                    # ---- wave 1: denominators (A-half sections first) ----
                    dchains = []
                    for it in range(4):
                        ps = psum1.tile([P, HALF], F32, tag="ps1", bufs=8)
                        dchains.append((it, ps))
                        for dd in range(HD):
                            nc.tensor.matmul(
                                ps[:],
                                (qT[:, dd, it * P : (it + 1) * P]),
                                (kt(dd)[:, 0:HALF]),
                                start=(dd == 0),
                                stop=False,
                            )
                    drchains = []
                    for it in range(4, 8):
                        ps = psum1.tile([P, HALF], F32, tag="ps1", bufs=8)
                        drchains.append((it, ps))
                        for t2 in range(HD // 2):
                            nc.tensor.matmul(
                                ps[:],
                                (q8[:, 2 * t2 : 2 * t2 + 2,
                                    (it - 4) * P : (it - 3) * P]),
                                (k8A[:, 2 * t2 : 2 * t2 + 2, :]),
                                start=(t2 == 0),
                                stop=False,
                                perf_mode=DR,
                            )
                    for it, ps in dchains:
                        for dd in range(HD, D // P):
                            nc.tensor.matmul(
                                ps[:],
                                (qT[:, dd, it * P : (it + 1) * P]),
                                (kt(dd)[:, 0:HALF]),
                                start=False,
                                stop=(dd == D // P - 1),
                            )
                        junk = scp.tile([P, HALF], BF16, tag="junk")
                        nc.scalar.activation(
                            junk[:], ps[:], EXP,
                            accum_out=dparts[:, it * NCORES + r : it * NCORES + r + 1],
                        )
                    for it, ps in drchains:
                        for t2 in range(HD // 2):
                            nc.tensor.matmul(
                                ps[:],
                                (q8[:, HD + 2 * t2 : HD + 2 * t2 + 2,
                                    (it - 4) * P : (it - 3) * P]),
                                (k8B[:, 2 * t2 : 2 * t2 + 2, :]),
                                start=False,
                                stop=(t2 == HD // 2 - 1),
                                perf_mode=DR,
                            )
                        junk = scp.tile([P, HALF], BF16, tag="junk")
                        nc.scalar.activation(
                            junk[:], ps[:], EXP, scale=SCALE2,
                            accum_out=dparts[:, it * NCORES + r : it * NCORES + r + 1],
                        )

                    # ---- wave 2: numerators ----
                    nchains = []
                    for jlo, ilo, e_sb in [(HALF, 0, eC), (0, HALF, eD)]:
                        for jt in range(4):
                            ps = psum1.tile([P, HALF], F32, tag="ps1", bufs=8)
                            nchains.append((jlo, ilo, e_sb, jt, ps))
                            for dd in range(HD):
                                nc.tensor.matmul(
                                    ps[:],
                                    (kt(dd)[:, jlo + jt * P : jlo + (jt + 1) * P]),
                                    (qT[:, dd, ilo : ilo + HALF]),
                                    start=(dd == 0),
                                    stop=False,
                                )
                    for jlo, ilo, e_sb, jt, ps in nchains:
                        for dd in range(HD, D // P):
                            nc.tensor.matmul(
                                ps[:],
                                (kt(dd)[:, jlo + jt * P : jlo + (jt + 1) * P]),
                                (qT[:, dd, ilo : ilo + HALF]),
                                start=False,
                                stop=(dd == D // P - 1),
                            )
                        nc.scalar.activation(
                            e_sb[:, r * 4 + jt], ps[:], EXP
                        )

            # ================= phase 2: numerator + normalize =================
            with (
                tc.tile_pool(name="vv", bufs=1) as vvp,
                tc.tile_pool(name="fin", bufs=3) as finp,
                tc.tile_pool(name="psum2", bufs=2, space="PSUM") as psum2,
                tc.tile_pool(name="psum2o", bufs=2, space="PSUM") as psum2o,
            ):
                for half_i, (e_sb, out_t) in enumerate([(eC, out_s), (eD, out_q)]):
                    vts = []
                    for r in range(NCORES):
                        vt = vvp.tile([P, 4, D], BF16, tag=f"v{r}",
                                      name=f"v{r}")
                        vts.append(vt)
                        nc.sync.dma_start(
                            vt[:],
                            _r(
                                v_all[
                                    r * ROWS + half_i * HALF : r * ROWS
                                    + half_i * HALF
                                    + HALF,
                                    :,
                                ]
                            ),
                        )
                    for it in range(4):
                        it_g = half_i * 4 + it
                        psA = psum2.tile([P, HALF], F32, tag="psA")
                        psB = psum2.tile([P, HALF], F32, tag="psB")
                        psO = psum2o.tile([P, 1], F32, tag="psO")
                        for j in range(32):
                            lhsT = e_sb[:, j, it * P : (it + 1) * P]
                            vt = vts[j // 4]
                            vj = j % 4
                            st = dict(start=(j == 0), stop=(j == 31))
                            nc.tensor.matmul(psA[:], lhsT, vt[:, vj, 0:HALF], **st)
                            nc.tensor.matmul(psB[:], lhsT, vt[:, vj, HALF:D], **st)
                            nc.tensor.matmul(psO[:], lhsT, ones_sb[:], **st)
                        dsum = finp.tile([P, 1], F32, tag="dsum")
                        nc.vector.reduce_sum(dsum[:], dparts[:, it_g * NCORES : (it_g + 1) * NCORES], axis=AX)
                        nc.vector.tensor_add(dsum[:], dsum[:], psO[:])
                        recip = finp.tile([P, 1], F32, tag="recip")
                        nc.vector.reciprocal(recip[:], dsum[:])
                        otile = finp.tile([P, D], F32, tag="otile")
                        nc.scalar.activation(
                            otile[:, 0:HALF], psA[:], COPY, scale=recip[:]
                        )
                        nc.scalar.activation(
                            otile[:, HALF:D], psB[:], COPY, scale=recip[:]
                        )
                        nc.sync.dma_start(
                            out_t[it * P : (it + 1) * P, :], otile[:]
                        )


_NC_CACHE = None


def _pack_wqk(wqkT):
    """[C, 2D] -> [128, 16*C] with contiguous per-partition runs."""
    return np.ascontiguousarray(
        wqkT.reshape(8, 128, 16, 128).transpose(1, 2, 0, 3).reshape(P, 16 * C)
    )


def kernel(query, s, TG_prompt, W_qkv, W_proj_s, W_proj_q):
    global _NC_CACHE
    import ml_dtypes

    BF = ml_dtypes.bfloat16
    query = np.asarray(query, dtype=np.float32)
    s = np.asarray(s, dtype=np.float32)
    TG_prompt = np.asarray(TG_prompt, dtype=np.float32)

    tT = np.ascontiguousarray((s + TG_prompt).T.astype(BF))
    sTf = np.ascontiguousarray(s.T.astype(BF))
    qryT = np.ascontiguousarray(query.T.astype(BF))
    wqkR = _pack_wqk(
        np.ascontiguousarray(np.asarray(W_qkv, np.float32)[: 2 * D].T)
    ).astype(BF)
    wpsT = np.ascontiguousarray(np.asarray(W_proj_s, np.float32).T.astype(BF))
    wpqT = np.ascontiguousarray(np.asarray(W_proj_q, np.float32).T.astype(BF))

    if _NC_CACHE is None:
        _NC_CACHE = build_program()
    nc = _NC_CACHE

    in_maps = []
    for m in range(NCORES):
        sl = slice(m * HALF, (m + 1) * HALF)
        in_maps.append(
            {
                "aT": np.ascontiguousarray(
                    np.concatenate([tT[:, sl], qryT[:, sl]], axis=1)
                ),
                "sT": np.ascontiguousarray(sTf[:, sl]),
                "wqkR": wqkR,
                "wpsT": wpsT,
                "wpqT": wpqT,
            }
        )

    res = run_bass_kernel_spmd(nc, in_maps, core_ids=list(range(NCORES)))
    outs = res.results

    x_s = np.concatenate([outs[m]["out_s"] for m in range(NCORES)], axis=0)
    x_q = np.concatenate([outs[m]["out_q"] for m in range(NCORES)], axis=0)
    return (x_s, x_q)
